# revision 23
# baseline (speedup 1.0000x reference)
"""Trainium2 Bass kernel for nn_AdditiveCoupling (NICE additive coupling layer).

reference math:
    first  = x[:, 0::2]            # (B, 392)
    second = x[:, 1::2]            # (B, 392)
    shift  = MLP(first)            # 392 -> 1000 -> (1000)x4 -> 392, ReLU between
    y[:, 0::2] = first
    y[:, 1::2] = second + shift    # i.e. y = x with shift added to odd columns
    returns (y, log_det_J)         # log_det_J passes through unchanged

Strategy: data-parallel over 8 NeuronCores. Each core takes a 512-row batch
shard and the full (replicated) weights; no inter-core communication.

Compute (per core): activations kept feature-major ([feat, batch]) so every
layer is matmul(psum[M,N] = W_chunk[K,M].T @ A[K,N]) with the weight chunk as
the stationary operand. Layer 1 runs bf16 (chunks of 98 over the 392-wide
contraction, k-outer with 8 concurrent PSUM-bank groups). The four hidden
layers and the output layer run fp8e4 with perf_mode=DoubleRow: operands are
[K=125, 2, *] — two K-tiles packed per PE cell (pairing features p and p+125
of each 250-wide chunk) for ~1.7x bf16 matmul throughput. PSUM eviction fuses
bias+ReLU on ScalarE and writes the paired-fp8 activation slices directly.
The last layer swaps operands (lhsT = activation chunk) so its output lands
batch-major [128, 392] in PSUM; its bias comes via a bf16 ones-row matmul
into the same accumulation group, and one strided VectorE add folds the shift
into x's odd columns in-place before the output DMA. Accuracy: y rel-l2 err
~5e-3 measured on-CPU for all-fp8 hidden layers (gate is 2e-2); the even
output columns are byte-exact x passthrough.

DMA: all weights stream as 0.5MB-read contiguous gpsimd SWDGE cast-DMAs
(fp32 DRAM -> fp8/bf16 SBUF inline); the read side measures ~300GB/s, near
the 358GB/s per-core HBM limit, which makes weight streaming (~62us) the
kernel's floor. x/outputs ride the two HWDGE queues. Weight slots are
quad-buffered so transfers never wait on slot release and the PE never
starves (a starved PE re-enters the 1.2GHz HAM-throttled clock state).
"""

import sys

sys.path.insert(0, "/opt/trn_rl_repo")

import numpy as np

import concourse.bass as bass  # noqa: F401  (engine types via nc)
import concourse.tile as tile
from concourse import bacc, mybir
from concourse.bass_utils import run_bass_kernel_spmd
from concourse.masks import make_identity

N_CORES = 8
B, D, MID = 4096, 784, 1000
HALF = D // 2  # 392
BS = B // N_CORES  # 512 rows per core
NB = BS // 128  # 4 batch tiles per core

KH = 98  # feature chunk for the 392-wide first contraction (4 chunks)
KM = 125  # feature half-chunk for the 1000-wide dims
NH = HALF // KH  # 4
NM = MID // KM  # 8 psum/eviction groups of 125
NC2 = NM // 2  # 4 paired (250-wide) fp8 contraction chunks
WPAD = 1024  # padded per-ko free width of paired weight tiles (step%16==0)
WOPAD = 400  # padded per-ko free width of paired W_out tiles

F32 = mybir.dt.float32
BF16 = mybir.dt.bfloat16
FP8 = mybir.dt.float8e4
DR = mybir.MatmulPerfMode.DoubleRow
RELU = mybir.ActivationFunctionType.Relu

_CACHED_NC = None

_SWDGE_QUEUES = ["qPoolDynamic", "qPoolDynamic1", "qPoolDynamic2", "qPoolDynamic3"]
_swdge_rr = [0]


def _gp_dma(nc, out, in_):
    inst = nc.gpsimd.dma_start(out=out, in_=in_)
    inst.ins.queue = _SWDGE_QUEUES[_swdge_rr[0] % len(_SWDGE_QUEUES)]
    _swdge_rr[0] += 1
    return inst


def build_nc():
    _swdge_rr[0] = 0
    nc = bacc.Bacc("TRN2", target_bir_lowering=False, debug=False, num_swdge_queues=4)

    x = nc.declare_dram_parameter("x", [BS, D], F32, isOutput=False)
    w_in = nc.declare_dram_parameter("W_in", [HALF, MID], F32, isOutput=False)
    b_in = nc.declare_dram_parameter("b_in", [MID], F32, isOutput=False)
    w_hid = nc.declare_dram_parameter("W_hid", [4, MID, MID], F32, isOutput=False)
    b_hid = nc.declare_dram_parameter("b_hid", [4, MID], F32, isOutput=False)
    w_out = nc.declare_dram_parameter("W_out", [MID, HALF], F32, isOutput=False)
    b_out = nc.declare_dram_parameter("b_out", [HALF], F32, isOutput=False)
    out = nc.declare_dram_parameter("out", [BS, D], F32, isOutput=True)

    with tile.TileContext(nc) as tc:
        with (
            tc.tile_pool(name="const", bufs=1) as constp,
            tc.tile_pool(name="xp", bufs=1) as xp,
            tc.tile_pool(name="winp", bufs=1) as winp,
            tc.tile_pool(name="whidp", bufs=4) as whidp,
            tc.tile_pool(name="woutp", bufs=1) as woutp,
            tc.tile_pool(name="actp", bufs=2) as actp,
            tc.tile_pool(name="biasp", bufs=1) as biasp,
            tc.tile_pool(name="psp", bufs=8, space="PSUM") as psp,
        ):
            ident = constp.tile([128, 128], F32, name="ident", tag="ident")
            make_identity(nc, ident)
            ones = constp.tile([1, 128], BF16, name="ones", tag="ones")
            nc.vector.memset(ones, 1.0)

            # ---- input: 4 tiles, alternating the two HWDGE queues ----
            xv = x.rearrange("(b p) d -> b p d", p=128)
            X = []
            x_eng = [nc.sync, nc.scalar, nc.sync, nc.scalar]
            for b in range(NB):
                xt = xp.tile([128, D], F32, name=f"x{b}", tag=f"x{b}")
                x_eng[b].dma_start(out=xt, in_=xv[b])
                X.append(xt)

            # ---- W_in: 4 bf16 cast-DMAs at the head of the SWDGE queue ----
            WIN = []
            for k in range(NH):
                wt = winp.tile([KH, MID], BF16, name=f"win{k}", tag=f"win{k}")
                _gp_dma(nc, wt, w_in[k * KH : (k + 1) * KH, :])
                WIN.append(wt)

            def load_whid(layer):
                """4 paired fp8 tiles [125, 2, WPAD]; (p, ko) holds W_hid row
                kc*250 + ko*125 + p — two contiguous 0.5MB-read cast-DMAs."""
                tiles = []
                for kc in range(NC2):
                    wt = whidp.tile(
                        [KM, 2, WPAD], FP8, name=f"wh{layer}_{kc}", tag=f"wh{kc}"
                    )
                    for ko in range(2):
                        r0 = kc * 250 + ko * KM
                        _gp_dma(nc, wt[:, ko, :MID], w_hid[layer, r0 : r0 + KM, :])
                    tiles.append(wt)
                return tiles

            WH0 = load_whid(0)

            bin_t = biasp.tile([KM, NM], F32, name="bin", tag="bin")
            nc.sync.dma_start(out=bin_t, in_=b_in.rearrange("(m p) -> p m", p=KM))
            bhid_t = []
            for i in range(4):
                bt = biasp.tile([KM, NM], F32, name=f"bh{i}", tag=f"bh{i}")
                nc.sync.dma_start(
                    out=bt, in_=b_hid[i].rearrange("(m p) -> p m", p=KM)
                )
                bhid_t.append(bt)

            # ---- split even columns + transpose to feature-major ----
            # A0[f] = first.T chunk f: [98 feats, 512 batch], bf16
            A0 = [
                actp.tile([KH, BS], BF16, name=f"A0_{f}", tag=f"a0_{f}")
                for f in range(NH)
            ]
            for b in range(NB):
                xb_pairs = X[b].rearrange("p (d two) -> p d two", two=2)
                for f in range(NH):
                    pt = psp.tile([KH, 128], F32, name=f"pt{b}_{f}", tag="bank")
                    nc.tensor.transpose(
                        pt, xb_pairs[:, f * KH : (f + 1) * KH, 0], ident
                    )
                    nc.vector.tensor_copy(A0[f][:, b * 128 : (b + 1) * 128], pt)

            def paired_act_tiles(name):
                return [
                    actp.tile([KM, 2, BS], FP8, name=f"A{name}_{c}", tag=f"a{c}")
                    for c in range(NC2)
                ]

            def evict(ps_m, m, A_next, bias_col, split=False):
                """bias+ReLU eviction of psum group m into its paired slice.
                With split=True, odd groups go to VectorE (tensor_scalar
                add-then-max) so the eviction wave runs on two engines —
                this bounds the tail after the last weight chunk lands."""
                dst = A_next[m // 2][:, m % 2, :]
                if split and m % 2 == 1:
                    nc.vector.tensor_scalar(
                        dst,
                        ps_m,
                        bias_col[:, m : m + 1],
                        0.0,
                        mybir.AluOpType.add,
                        mybir.AluOpType.max,
                    )
                else:
                    nc.scalar.activation(
                        dst, ps_m, RELU, bias=bias_col[:, m : m + 1]
                    )

            # ---- layer 1: 392 -> 1000, bf16, k-outer; paired-fp8 output ----
            A1 = paired_act_tiles("1")
            ps = [
                psp.tile([KM, BS], F32, name=f"ps1_{m}", tag="bank")
                for m in range(NM)
            ]
            for k in range(NH):
                for m in range(NM):
                    nc.tensor.matmul(
                        ps[m],
                        WIN[k][:, m * KM : (m + 1) * KM],
                        A0[k],
                        start=(k == 0),
                        stop=(k == NH - 1),
                    )
            for m in range(NM):
                evict(ps[m], m, A1, bin_t)

            # ---- hidden layers: 1000 -> 1000, fp8 DoubleRow, k-outer ----
            # W_out is streamed BEFORE wh3: the last weights on the wire then
            # feed h3, whose per-chunk k-passes overlap the stream, leaving
            # only a short eviction+output chain after the final weight byte.
            A_prev = A1
            WH = WH0
            WOUT = None
            for layer in range(4):
                if layer == 2:
                    WOUT = []
                    for kc in range(NC2):
                        wt = woutp.tile(
                            [KM, 2, WOPAD], FP8, name=f"wo{kc}", tag=f"wo{kc}"
                        )
                        for ko in range(2):
                            r0 = kc * 250 + ko * KM
                            _gp_dma(
                                nc, wt[:, ko, :HALF], w_out[r0 : r0 + KM, :]
                            )
                        WOUT.append(wt)
                    bout_t = biasp.tile([1, HALF], BF16, name="bout", tag="bout")
                    _gp_dma(nc, bout_t, b_out[:].unsqueeze(0))
                if layer < 3:
                    WH_next = load_whid(layer + 1)
                else:
                    WH_next = None

                A_next = paired_act_tiles(f"h{layer}")
                bias_col = bhid_t[layer]
                psh = [
                    psp.tile([KM, BS], F32, name=f"psh{layer}_{m}", tag="bank")
                    for m in range(NM)
                ]
                for kc in range(NC2):
                    for m in range(NM):
                        nc.tensor.matmul(
                            psh[m],
                            WH[kc][:, :, m * KM : (m + 1) * KM],
                            A_prev[kc][:, :, :],
                            start=(kc == 0),
                            stop=(kc == NC2 - 1),
                            perf_mode=DR,
                        )
                for m in range(NM):
                    evict(psh[m], m, A_next, bias_col, split=(layer == 3))
                A_prev = A_next
                WH = WH_next

            # ---- output layer: 1000 -> 392, fp8 DoubleRow, batch-major ----
            ov = out.rearrange("(b p) d -> b p d", p=128)
            PSO = [
                psp.tile([128, HALF], F32, name=f"pso{b}", tag="bank")
                for b in range(NB)
            ]
            for kc in range(NC2):
                for b in range(NB):
                    nc.tensor.matmul(
                        PSO[b],
                        A_prev[kc][:, :, b * 128 : (b + 1) * 128],
                        WOUT[kc][:, :, :HALF],
                        start=(kc == 0),
                        stop=False,
                        perf_mode=DR,
                    )
            for b in range(NB):
                # bias via bf16 ones-row into the same accumulation group
                nc.tensor.matmul(PSO[b], ones, bout_t, start=False, stop=True)
                xb_odd = X[b].rearrange("p (d two) -> p d two", two=2)[:, :, 1]
                nc.vector.tensor_add(xb_odd, xb_odd, PSO[b])
                eng = nc.sync if b % 2 == 0 else nc.scalar
                eng.dma_start(out=ov[b], in_=X[b])

    nc.finalize()  # Bacc register allocation + freeze (bass2jax won't do it)
    return nc


def get_nc():
    global _CACHED_NC
    if _CACHED_NC is None:
        _CACHED_NC = build_nc()
    return _CACHED_NC


def run(inputs, trace=False, tmpdir=None):
    nc = get_nc()
    f32c = lambda a: np.ascontiguousarray(np.asarray(a), dtype=np.float32)
    x = f32c(inputs["x"])
    shared = {
        k: f32c(inputs[k])
        for k in ("W_in", "b_in", "W_hid", "b_hid", "W_out", "b_out")
    }
    in_maps = [
        {"x": x[i * BS : (i + 1) * BS], **shared} for i in range(N_CORES)
    ]
    res = run_bass_kernel_spmd(
        nc, in_maps, core_ids=list(range(N_CORES)), trace=trace, tmpdir=tmpdir
    )
    y = np.concatenate([res.results[i]["out"] for i in range(N_CORES)], axis=0)
    return y, res


def kernel(**inputs):
    y, _ = run(inputs)
    log_det_J = np.asarray(inputs["log_det_J"], dtype=np.float32)
    return y, log_det_J


# revision 24
# speedup vs baseline: 1.0505x; 1.0505x over previous
"""Trainium2 Bass kernel for nn_AdditiveCoupling (NICE additive coupling layer).

reference math:
    first  = x[:, 0::2]            # (B, 392)
    second = x[:, 1::2]            # (B, 392)
    shift  = MLP(first)            # 392 -> 1000 -> (1000)x4 -> 392, ReLU between
    y[:, 0::2] = first
    y[:, 1::2] = second + shift    # i.e. y = x with shift added to odd columns
    returns (y, log_det_J)         # log_det_J passes through unchanged

Strategy: data-parallel over 8 NeuronCores. Each core takes a 512-row batch
shard and the full (replicated) weights; no inter-core communication.

Compute (per core): activations kept feature-major ([feat, batch]) so every
layer is matmul(psum[M,N] = W_chunk[K,M].T @ A[K,N]) with the weight chunk as
the stationary operand. Layer 1 runs bf16 (chunks of 98 over the 392-wide
contraction, k-outer with 8 concurrent PSUM-bank groups). The four hidden
layers and the output layer run fp8e4 with perf_mode=DoubleRow: operands are
[K=125, 2, *] — two K-tiles packed per PE cell (pairing features p and p+125
of each 250-wide chunk) for ~1.7x bf16 matmul throughput. PSUM eviction fuses
bias+ReLU on ScalarE and writes the paired-fp8 activation slices directly.
The last layer swaps operands (lhsT = activation chunk) so its output lands
batch-major [128, 392] in PSUM; its bias comes via a bf16 ones-row matmul
into the same accumulation group, and one strided VectorE add folds the shift
into x's odd columns in-place before the output DMA. Accuracy: y rel-l2 err
~5e-3 measured on-CPU for all-fp8 hidden layers (gate is 2e-2); the even
output columns are byte-exact x passthrough.

DMA: all weights stream as 0.5MB-read contiguous gpsimd SWDGE cast-DMAs
(fp32 DRAM -> fp8/bf16 SBUF inline); the read side measures ~300GB/s, near
the 358GB/s per-core HBM limit, which makes weight streaming (~62us) the
kernel's floor. x/outputs ride the two HWDGE queues. Weight slots are
quad-buffered so transfers never wait on slot release and the PE never
starves (a starved PE re-enters the 1.2GHz HAM-throttled clock state).
"""

import sys

sys.path.insert(0, "/opt/trn_rl_repo")

import numpy as np

import concourse.bass as bass  # noqa: F401  (engine types via nc)
import concourse.tile as tile
from concourse import bacc, mybir
from concourse.bass_utils import run_bass_kernel_spmd
from concourse.masks import make_identity

N_CORES = 8
B, D, MID = 4096, 784, 1000
HALF = D // 2  # 392
BS = B // N_CORES  # 512 rows per core
NB = BS // 128  # 4 batch tiles per core

KH = 98  # feature chunk for the 392-wide first contraction (4 chunks)
KM = 125  # feature half-chunk for the 1000-wide dims
NH = HALF // KH  # 4
NM = MID // KM  # 8 psum/eviction groups of 125
NC2 = NM // 2  # 4 paired (250-wide) fp8 contraction chunks
WPAD = 1024  # padded per-ko free width of paired weight tiles (step%16==0)
WOPAD = 400  # padded per-ko free width of paired W_out tiles

F32 = mybir.dt.float32
BF16 = mybir.dt.bfloat16
FP8 = mybir.dt.float8e4
DR = mybir.MatmulPerfMode.DoubleRow
RELU = mybir.ActivationFunctionType.Relu

_CACHED_NC = None

_SWDGE_QUEUES = ["qPoolDynamic", "qPoolDynamic1", "qPoolDynamic2", "qPoolDynamic3"]
_swdge_rr = [0]


def _gp_dma(nc, out, in_):
    inst = nc.gpsimd.dma_start(out=out, in_=in_)
    inst.ins.queue = _SWDGE_QUEUES[_swdge_rr[0] % len(_SWDGE_QUEUES)]
    _swdge_rr[0] += 1
    return inst


def build_nc():
    _swdge_rr[0] = 0
    nc = bacc.Bacc("TRN2", target_bir_lowering=False, debug=False, num_swdge_queues=4)

    x = nc.declare_dram_parameter("x", [BS, D], F32, isOutput=False)
    w_in = nc.declare_dram_parameter("W_in", [HALF, MID], F32, isOutput=False)
    b_in = nc.declare_dram_parameter("b_in", [MID], F32, isOutput=False)
    w_hid = nc.declare_dram_parameter("W_hid", [4, MID, MID], F32, isOutput=False)
    b_hid = nc.declare_dram_parameter("b_hid", [4, MID], F32, isOutput=False)
    w_out = nc.declare_dram_parameter("W_out", [MID, HALF], F32, isOutput=False)
    b_out = nc.declare_dram_parameter("b_out", [HALF], F32, isOutput=False)
    out = nc.declare_dram_parameter("out", [BS, D], F32, isOutput=True)

    with tile.TileContext(nc) as tc:
        with (
            tc.tile_pool(name="const", bufs=1) as constp,
            tc.tile_pool(name="xp", bufs=1) as xp,
            tc.tile_pool(name="winp", bufs=1) as winp,
            tc.tile_pool(name="whidp", bufs=4) as whidp,
            tc.tile_pool(name="woutp", bufs=1) as woutp,
            tc.tile_pool(name="actp", bufs=2) as actp,
            tc.tile_pool(name="biasp", bufs=1) as biasp,
            tc.tile_pool(name="psp", bufs=8, space="PSUM") as psp,
        ):
            ident = constp.tile([128, 128], F32, name="ident", tag="ident")
            make_identity(nc, ident)
            ones = constp.tile([1, 128], BF16, name="ones", tag="ones")
            nc.vector.memset(ones, 1.0)

            # ---- input: 4 tiles, alternating the two HWDGE queues ----
            xv = x.rearrange("(b p) d -> b p d", p=128)
            X = []
            x_eng = [nc.sync, nc.scalar, nc.sync, nc.scalar]
            for b in range(NB):
                xt = xp.tile([128, D], F32, name=f"x{b}", tag=f"x{b}")
                x_eng[b].dma_start(out=xt, in_=xv[b])
                X.append(xt)

            # ---- W_in: 4 bf16 cast-DMAs at the head of the SWDGE queue ----
            WIN = []
            for k in range(NH):
                wt = winp.tile([KH, MID], BF16, name=f"win{k}", tag=f"win{k}")
                _gp_dma(nc, wt, w_in[k * KH : (k + 1) * KH, :])
                WIN.append(wt)

            def load_whid(layer):
                """4 paired fp8 tiles [125, 2, WPAD]; (p, ko) holds W_hid row
                kc*250 + ko*125 + p — two contiguous 0.5MB-read cast-DMAs."""
                tiles = []
                for kc in range(NC2):
                    wt = whidp.tile(
                        [KM, 2, WPAD], FP8, name=f"wh{layer}_{kc}", tag=f"wh{kc}"
                    )
                    for ko in range(2):
                        r0 = kc * 250 + ko * KM
                        _gp_dma(nc, wt[:, ko, :MID], w_hid[layer, r0 : r0 + KM, :])
                    tiles.append(wt)
                return tiles

            WH0 = load_whid(0)

            bin_t = biasp.tile([KM, NM], F32, name="bin", tag="bin")
            nc.sync.dma_start(out=bin_t, in_=b_in.rearrange("(m p) -> p m", p=KM))
            bhid_t = []
            for i in range(4):
                bt = biasp.tile([KM, NM], F32, name=f"bh{i}", tag=f"bh{i}")
                nc.sync.dma_start(
                    out=bt, in_=b_hid[i].rearrange("(m p) -> p m", p=KM)
                )
                bhid_t.append(bt)

            # ---- split even columns + transpose to feature-major ----
            # A0[f] = first.T chunk f: [98 feats, 512 batch], bf16
            A0 = [
                actp.tile([KH, BS], BF16, name=f"A0_{f}", tag=f"a0_{f}")
                for f in range(NH)
            ]
            for b in range(NB):
                xb_pairs = X[b].rearrange("p (d two) -> p d two", two=2)
                for f in range(NH):
                    pt = psp.tile([KH, 128], F32, name=f"pt{b}_{f}", tag="bank")
                    nc.tensor.transpose(
                        pt, xb_pairs[:, f * KH : (f + 1) * KH, 0], ident
                    )
                    nc.vector.tensor_copy(A0[f][:, b * 128 : (b + 1) * 128], pt)

            def paired_act_tiles(name):
                return [
                    actp.tile([KM, 2, BS], FP8, name=f"A{name}_{c}", tag=f"a{c}")
                    for c in range(NC2)
                ]

            def evict(ps_m, m, A_next, bias_col, split=False):
                """bias+ReLU eviction of psum group m into its paired slice.
                With split=True, odd groups go to VectorE (tensor_scalar
                add-then-max) so the eviction wave runs on two engines —
                this bounds the tail after the last weight chunk lands."""
                dst = A_next[m // 2][:, m % 2, :]
                if split and m % 2 == 1:
                    nc.vector.tensor_scalar(
                        dst,
                        ps_m,
                        bias_col[:, m : m + 1],
                        0.0,
                        mybir.AluOpType.add,
                        mybir.AluOpType.max,
                    )
                else:
                    nc.scalar.activation(
                        dst, ps_m, RELU, bias=bias_col[:, m : m + 1]
                    )

            # ---- layer 1: 392 -> 1000, bf16, k-outer; paired-fp8 output ----
            A1 = paired_act_tiles("1")
            ps = [
                psp.tile([KM, BS], F32, name=f"ps1_{m}", tag="bank")
                for m in range(NM)
            ]
            for k in range(NH):
                for m in range(NM):
                    nc.tensor.matmul(
                        ps[m],
                        WIN[k][:, m * KM : (m + 1) * KM],
                        A0[k],
                        start=(k == 0),
                        stop=(k == NH - 1),
                    )
            for m in range(NM):
                evict(ps[m], m, A1, bin_t)

            # ---- hidden layers: 1000 -> 1000, fp8 DoubleRow, k-outer ----
            # W_out is streamed BEFORE wh3: the last weights on the wire then
            # feed h3, whose per-chunk k-passes overlap the stream, leaving
            # only a short eviction+output chain after the final weight byte.
            A_prev = A1
            WH = WH0
            WOUT = None
            for layer in range(4):
                if layer < 3:
                    WH_next = load_whid(layer + 1)
                else:
                    WH_next = None
                    WOUT = []
                    for kc in range(NC2):
                        wt = woutp.tile(
                            [KM, 2, WOPAD], FP8, name=f"wo{kc}", tag=f"wo{kc}"
                        )
                        for ko in range(2):
                            r0 = kc * 250 + ko * KM
                            _gp_dma(
                                nc, wt[:, ko, :HALF], w_out[r0 : r0 + KM, :]
                            )
                        WOUT.append(wt)
                    bout_t = biasp.tile([1, HALF], BF16, name="bout", tag="bout")
                    _gp_dma(nc, bout_t, b_out[:].unsqueeze(0))

                A_next = paired_act_tiles(f"h{layer}")
                bias_col = bhid_t[layer]
                psh = [
                    psp.tile([KM, BS], F32, name=f"psh{layer}_{m}", tag="bank")
                    for m in range(NM)
                ]
                for kc in range(NC2):
                    for m in range(NM):
                        nc.tensor.matmul(
                            psh[m],
                            WH[kc][:, :, m * KM : (m + 1) * KM],
                            A_prev[kc][:, :, :],
                            start=(kc == 0),
                            stop=(kc == NC2 - 1),
                            perf_mode=DR,
                        )
                for m in range(NM):
                    evict(psh[m], m, A_next, bias_col, split=(layer == 3))
                A_prev = A_next
                WH = WH_next

            # ---- output layer: 1000 -> 392, fp8 DoubleRow, batch-major ----
            ov = out.rearrange("(b p) d -> b p d", p=128)
            PSO = [
                psp.tile([128, HALF], F32, name=f"pso{b}", tag="bank")
                for b in range(NB)
            ]
            for kc in range(NC2):
                for b in range(NB):
                    nc.tensor.matmul(
                        PSO[b],
                        A_prev[kc][:, :, b * 128 : (b + 1) * 128],
                        WOUT[kc][:, :, :HALF],
                        start=(kc == 0),
                        stop=False,
                        perf_mode=DR,
                    )
            for b in range(NB):
                # bias via bf16 ones-row into the same accumulation group
                nc.tensor.matmul(PSO[b], ones, bout_t, start=False, stop=True)
                xb_odd = X[b].rearrange("p (d two) -> p d two", two=2)[:, :, 1]
                nc.vector.tensor_add(xb_odd, xb_odd, PSO[b])
                eng = nc.sync if b % 2 == 0 else nc.scalar
                eng.dma_start(out=ov[b], in_=X[b])

    nc.finalize()  # Bacc register allocation + freeze (bass2jax won't do it)
    return nc


def get_nc():
    global _CACHED_NC
    if _CACHED_NC is None:
        _CACHED_NC = build_nc()
    return _CACHED_NC


def run(inputs, trace=False, tmpdir=None):
    nc = get_nc()
    f32c = lambda a: np.ascontiguousarray(np.asarray(a), dtype=np.float32)
    x = f32c(inputs["x"])
    shared = {
        k: f32c(inputs[k])
        for k in ("W_in", "b_in", "W_hid", "b_hid", "W_out", "b_out")
    }
    in_maps = [
        {"x": x[i * BS : (i + 1) * BS], **shared} for i in range(N_CORES)
    ]
    res = run_bass_kernel_spmd(
        nc, in_maps, core_ids=list(range(N_CORES)), trace=trace, tmpdir=tmpdir
    )
    y = np.concatenate([res.results[i]["out"] for i in range(N_CORES)], axis=0)
    return y, res


def kernel(**inputs):
    y, _ = run(inputs)
    log_det_J = np.asarray(inputs["log_det_J"], dtype=np.float32)
    return y, log_det_J
